# revision 56
# baseline (speedup 1.0000x reference)
"""DenseCapsule routing kernel for 8 trn2 NeuronCores (Bass/Tile).

Problem (fp32 inputs):
  x:      [B=32, IN_N=2048, K=16]
  weight: [O=64, IN_N=2048, D=32, K=16]
  x_hat  = einsum('oidk,bik->boid', weight, x)
  3 rounds of dynamic routing (softmax over o, squash over d)
  out:    [B=32, O=64, D=32]

Sharding: by input capsule i (256 per core).  Softmax over o and squash
over d are then core-local; only the [B,O,D] partial sums s_t need an
AllReduce (262KB, 3x).

Per-core layouts (partition dim first):
  w1[g]   [128=(8 i_lo,16 k), 2048=(64 o,32 d)]  g in [0,32) igrps of 8 i
  x_bd[g] [128=(8 i_lo,16 k), 256=(32 b, 8 i')]  block diag: nonzero iff i_lo==i'
  x_ik    [128=(8 i_lo,16 k), (32 g, 32 b)]      dense x columns
  xh[m]   [128=(4 o_lo,32 d), (4 g,32 b,8 i')]   x_hat for o-chunk m=o//4, per quad
  beta/e/c [128=(4 b_lo,32 i_loc), (8 bq,64 o)]  b = 4*bq + b_lo, i_loc = 8*g_lo+i_lo
  crep/y  [128=(8 i_lo,16 k), ... 64 o]          c replicated over k
  s/u/v   [128=(4 o_lo,32 d), (16 og,32 b)]      o = 4*og + o_lo
"""

import numpy as np

B, O, IN_N, D, K = 32, 64, 2048, 32, 16
NCORES = 8
I_CORE = IN_N // NCORES          # 256
NG = I_CORE // 8                 # 32 igrps of 8 i
NQ = NG // 4                     # 8 quads of 4 igrps (32 i)
ROUTINGS = 3

_CACHE = {}


def _bf16(a):
    import ml_dtypes
    return np.asarray(a, dtype=ml_dtypes.bfloat16)


def _prep_core_inputs(x, weight, core):
    """Build the per-core DRAM input arrays (numpy, host side)."""
    i0 = core * I_CORE
    wc = weight[:, i0:i0 + I_CORE]                    # [64, 256, 32, 16]
    # w1[g, i_lo*16+k, o*32+d]
    w1 = np.ascontiguousarray(wc.transpose(1, 3, 0, 2))   # [256 i, 16 k, 64 o, 32 d]
    w1 = w1.reshape(NG, 8, K, O * D).reshape(NG, 128, O * D)

    xc = x[:, i0:i0 + I_CORE, :]                      # [32 b, 256 i, 16 k]
    xr = xc.reshape(B, NG, 8, K)                      # [b, g, i_lo, k]

    x_bd = np.zeros((NG, 8, K, B, 8), dtype=np.float32)
    for j in range(8):
        # x_bd[g, j, k, b, j] = x[b, i0+8g+j, k]
        x_bd[:, j, :, :, j] = xr[:, :, j, :].transpose(1, 2, 0)
    # -> [128, NG*256] so the partition dim is first
    x_bd = x_bd.reshape(NG, 128, B * 8).transpose(1, 0, 2).reshape(128, NG * B * 8)

    # x_ik[i_lo*16+k, g, b]
    x_ik = np.ascontiguousarray(xr.transpose(2, 3, 1, 0)).reshape(128, NG * B)

    return {"w1": _bf16(w1), "x_bd": _bf16(x_bd), "x_ik": _bf16(x_ik)}


def _prep_static():
    # rep_sel[p, (j*4+g_lo)*128 + i_lo*16 + k] = (p == 32*j + 8*g_lo + i_lo)
    # full-K selector: quadrant masking baked in so matmul inputs can sit
    # at partition base 0 (non-zero input base partitions hang real HW).
    p = np.arange(128)[:, None]
    col = np.arange(16 * 128)[None, :]
    blk, rem = col // 128, col % 128
    j, g_lo = blk // 4, blk % 4
    i_lo = rem // K
    rep_sel = (p == (32 * j + 8 * g_lo + i_lo)).astype(np.float32)

    # mask4[p, j] = (p//32 == j);  rep4 = mask4.T
    j = np.arange(4)[None, :]
    mask4 = ((p // 32) == j).astype(np.float32)
    rep4 = np.ascontiguousarray(mask4.T)
    zeros128 = np.zeros((128, 128), dtype=np.float32)
    return {"rep_sel": _bf16(rep_sel), "mask4": mask4, "rep4": rep4,
            "zeros128": _bf16(zeros128)}


def build_nc(n_cores=NCORES, n_routings=ROUTINGS, skip_cc=False, stage=5):
    import concourse.bass as bass
    import concourse.bacc as bacc
    import concourse.tile as tile
    import concourse.mybir as mybir
    from contextlib import ExitStack

    dt = mybir.dt
    AF = mybir.ActivationFunctionType
    ALU = mybir.AluOpType

    nc = bacc.Bacc("TRN2", target_bir_lowering=False, debug=False,
                   num_devices=n_cores)

    w1_d = nc.dram_tensor("w1", [NG, 128, O * D], dt.bfloat16, kind="ExternalInput")
    xbd_d = nc.dram_tensor("x_bd", [128, NG * B * 8], dt.bfloat16, kind="ExternalInput")
    xik_d = nc.dram_tensor("x_ik", [128, NG * B], dt.bfloat16, kind="ExternalInput")
    rsel_d = nc.dram_tensor("rep_sel", [128, 2048], dt.bfloat16, kind="ExternalInput")
    m4_d = nc.dram_tensor("mask4", [128, 4], dt.float32, kind="ExternalInput")
    r4_d = nc.dram_tensor("rep4", [4, 128], dt.float32, kind="ExternalInput")
    z128_d = nc.dram_tensor("zeros128", [128, 128], dt.bfloat16, kind="ExternalInput")
    out_d = nc.dram_tensor("out_raw", [128, 512], dt.float32, kind="ExternalOutput")

    with tile.TileContext(nc) as tc, ExitStack() as ctx:
        wp = ctx.enter_context(tc.tile_pool(name="w1", bufs=NG))
        cst = ctx.enter_context(tc.tile_pool(name="cst", bufs=1))
        xhp = ctx.enter_context(tc.tile_pool(name="xh", bufs=4))
        yp = ctx.enter_context(tc.tile_pool(name="y", bufs=2))
        ecp = ctx.enter_context(tc.tile_pool(name="ec", bufs=2))
        crp = ctx.enter_context(tc.tile_pool(name="crep", bufs=2))
        smp = ctx.enter_context(tc.tile_pool(name="small", bufs=1))
        up = ctx.enter_context(tc.tile_pool(name="uacc", bufs=1))
        gen_ps = ctx.enter_context(tc.tile_pool(name="genps", bufs=2, space="PSUM"))
        beta_ps = ctx.enter_context(tc.tile_pool(name="betaps", bufs=2, space="PSUM"))
        crep_ps = ctx.enter_context(tc.tile_pool(name="crepps", bufs=1, space="PSUM"))
        s_ps = ctx.enter_context(tc.tile_pool(name="sps", bufs=1, space="PSUM"))
        dram = ctx.enter_context(tc.tile_pool(name="dram", bufs=2, space="DRAM"))

        # ---- load inputs ----
        w1 = []
        for g in range(NG):
            t = wp.tile([128, O * D], dt.bfloat16, tag="w1")
            nc.sync.dma_start(t[:], w1_d[g])
            w1.append(t)
        x_bd = cst.tile([128, NG * B * 8], dt.bfloat16, tag="xbd")
        nc.sync.dma_start(x_bd[:], xbd_d[:])
        x_ik = cst.tile([128, NG * B], dt.bfloat16, tag="xik")
        nc.sync.dma_start(x_ik[:], xik_d[:])
        rep_sel = cst.tile([128, 2048], dt.bfloat16, tag="rsel")
        nc.sync.dma_start(rep_sel[:], rsel_d[:])
        mask4 = cst.tile([128, 4], dt.float32, tag="m4")
        nc.sync.dma_start(mask4[:], m4_d[:])
        rep4 = cst.tile([4, 128], dt.float32, tag="r4")
        nc.sync.dma_start(rep4[:], r4_d[:])
        zeros128 = cst.tile([128, 128], dt.bfloat16, tag="z128")
        nc.sync.dma_start(zeros128[:], z128_d[:])

        def bank_bracket(ps_tile, which):
            """Full-partition x0 matmul to open/close a psum bank group."""
            nc.tensor.matmul(ps_tile[:, 0:512], zeros128[:], x_bd[:, 0:512],
                             start=(which == "open"), stop=(which == "close"))

        u_sb = up.tile([128, 512], dt.float32, tag="u")
        u_blk = up.tile([128, 2048], dt.bfloat16, tag="ublk")

        def s_out_ap(s_tile, o):
            return s_tile[32 * (o % 4):32 * (o % 4) + 32,
                          (o // 4) * 32:(o // 4) * 32 + 32]

        def s_tile_pos(o):
            return (0, 32 * (o % 4))

        def allreduce_squash(s_tile, t):
            """AllReduce psum s partials, squash, update u; return v tile."""
            s_sb = smp.tile([128, 512], dt.float32, tag="s_sb")
            nc.scalar.copy(s_sb[:], s_tile[:])
            bi = dram.tile([128, 512], dt.float32, tag="cc_in")
            bo = dram.tile([128, 512], dt.float32, tag="cc_out")
            nc.sync.dma_start(bi[:], s_sb[:])
            if skip_cc:
                nc.sync.dma_start(bo[:], bi[:])
            else:
                nc.gpsimd.collective_compute(
                    "AllReduce", ALU.add,
                    replica_groups=[list(range(n_cores))],
                    ins=[bi.opt()], outs=[bo.opt()],
                )
            s_full = smp.tile([128, 512], dt.float32, tag="s_full")
            nc.sync.dma_start(s_full[:], bo[:])

            alpha = (1.0 / O) if t == 0 else 1.0
            s2 = smp.tile([128, 512], dt.float32, tag="s_sb")
            nc.scalar.activation(s2[:], s_full[:], AF.Square, scale=alpha)
            n2 = beta_ps.tile([4, 512], dt.float32, tag="beta")
            nc.tensor.matmul(n2[:], mask4[:], s2[:], start=True, stop=True)
            n2p1 = smp.tile([4, 512], dt.float32, tag="sqA")
            nc.vector.tensor_scalar_add(n2p1[:], n2[:], 1.0)
            r1 = smp.tile([4, 512], dt.float32, tag="sqB")
            nc.vector.reciprocal(r1[:], n2p1[:])
            t2 = smp.tile([4, 512], dt.float32, tag="sqC")
            nc.vector.tensor_mul(t2[:], n2[:], r1[:])
            nrm = smp.tile([4, 512], dt.float32, tag="sqA")
            nc.scalar.activation(nrm[:], n2[:], AF.Sqrt)
            nrmp = smp.tile([4, 512], dt.float32, tag="sqB")
            nc.vector.tensor_scalar_add(nrmp[:], nrm[:], 1e-8)
            r2 = smp.tile([4, 512], dt.float32, tag="sqA")
            nc.vector.reciprocal(r2[:], nrmp[:])
            sc = smp.tile([4, 512], dt.float32, tag="sqB")
            nc.vector.tensor_mul(sc[:], t2[:], r2[:])
            if alpha != 1.0:
                nc.vector.tensor_scalar_mul(sc[:], sc[:], alpha)
            srep = beta_ps.tile([128, 512], dt.float32, tag="beta")
            nc.tensor.matmul(srep[:], rep4[:], sc[:], start=True, stop=True)
            v = smp.tile([128, 512], dt.float32, tag="v")
            nc.vector.tensor_mul(v[:], s_full[:], srep[:])

            if t == 0:
                nc.vector.tensor_copy(u_sb[:], v[:])
            else:
                nc.vector.tensor_add(u_sb[:], u_sb[:], v[:])
            if t < n_routings - 1:
                # u_blk[p, (og, b, j)] = u[p, (og, b)] * mask4[p, j]
                u3 = u_sb[:].rearrange("p (og b) -> p og b", og=16)
                u4 = u3.unsqueeze(3).broadcast_to([128, 16, 32, 4])
                m2 = mask4[:].rearrange("p j -> p j")
                m4b = m2.unsqueeze(1).unsqueeze(1).broadcast_to([128, 16, 32, 4])
                ub = u_blk[:].rearrange("p (og b j) -> p og b j", og=16, b=32)
                nc.vector.tensor_tensor(ub, u4, m4b, ALU.mult)
            return v

        # ================= iteration 0 =================
        v = None
        s0 = s_ps.tile([128, 512], dt.float32, tag="s")
        bank_bracket(s0, "open")
        for g in range(NG):
            rhs = x_ik[:, g * B:(g + 1) * B]
            for o in range(O):
                nc.tensor.matmul(
                    s_out_ap(s0, o), w1[g][:, o * D:(o + 1) * D], rhs,
                    start=False, stop=False,
                    tile_position=s_tile_pos(o))
        bank_bracket(s0, "close")
        v = allreduce_squash(s0, 0)

        # ================= iterations 1, 2 =================
        for t in range(1, n_routings):
            s_t = s_ps.tile([128, 512], dt.float32, tag="s")
            bank_bracket(s_t, "open")
            for q in range(NQ):
                # --- generate x_hat (o-chunk m at a time) + beta pieces ---
                bps = beta_ps.tile([128, 512], dt.float32, tag="beta")
                bank_bracket(bps, "open")
                for m in range(16):
                    ps = gen_ps.tile([128, 1024], dt.float32, tag="gen")
                    for gi in range(4):
                        g = q * 4 + gi
                        # 2 banks, 4 quarter writes: start on each bank's
                        # first write, stop on its last.
                        nc.tensor.matmul(
                            ps[:, gi * 256:(gi + 1) * 256],
                            w1[g][:, m * 128:(m + 1) * 128],
                            x_bd[:, g * 256:(g + 1) * 256],
                            start=(gi % 2 == 0), stop=(gi % 2 == 1))
                    # drain reorders cols (g,b,i') -> (b,g,i') so each b's
                    # 32 i-cols are contiguous (walrus: lhsT AP one free dim)
                    xh = xhp.tile([128, 1024], dt.bfloat16, tag="xh")
                    ps_v = ps[:].rearrange("p (g b i) -> p g b i", g=4, b=32)
                    xh_v = xh[:].rearrange("p (b g i) -> p g b i", b=32, g=4)
                    if m % 2 == 0:
                        nc.vector.tensor_copy(xh_v, ps_v)
                    else:
                        nc.scalar.copy(xh_v, ps_v)

                    # beta: [32 i, 4 o] pieces, contraction over (o_lo, d)
                    for b in range(B if stage >= 2 else 0):
                        lhsT = xh[:, b * 32:(b + 1) * 32]  # [128, 32]
                        rhs = u_blk[:, (m * 32 + b) * 4:(m * 32 + b) * 4 + 4]
                        j, bq = b % 4, b // 4
                        nc.tensor.matmul(
                            bps[32 * j:32 * j + 32,
                                bq * 64 + 4 * m:bq * 64 + 4 * m + 4],
                            lhsT, rhs,
                            start=False, stop=False,
                            tile_position=(0, 32 * j))

                bank_bracket(bps, "close")

                # --- softmax over o (free dim) ---
                e_sb = ecp.tile([128, 512], dt.bfloat16, tag="e")
                z = smp.tile([128, 8], dt.float32, tag="z")
                for bq in range(8 if stage >= 3 else 0):
                    nc.scalar.activation(
                        e_sb[:, bq * 64:(bq + 1) * 64],
                        bps[:, bq * 64:(bq + 1) * 64],
                        AF.Exp, accum_out=z[:, bq:bq + 1])
                zr = smp.tile([128, 8], dt.float32, tag="zr")
                if stage >= 3:
                    nc.vector.reciprocal(zr[:], z[:])
                c_sb = ecp.tile([128, 512], dt.bfloat16, tag="c")
                for bq in range(8 if stage >= 3 else 0):
                    nc.vector.tensor_scalar_mul(
                        c_sb[:, bq * 64:(bq + 1) * 64],
                        e_sb[:, bq * 64:(bq + 1) * 64], zr[:, bq:bq + 1])

                # --- replicate c over k; y = c * x;  s += W @ y ---
                for g_lo in range(4 if stage >= 4 else 0):
                    g = q * 4 + g_lo
                    y = yp.tile([128, B * 64], dt.bfloat16, tag="y")
                    for fs in range(4):
                        b0 = fs * 8
                        cps = crep_ps.tile([128, 512], dt.float32, tag="crep")
                        for pi in range(8):
                            b = b0 + pi
                            j, bq = b % 4, b // 4
                            sel = rep_sel[:, (j * 4 + g_lo) * 128:
                                          (j * 4 + g_lo) * 128 + 128]
                            nc.tensor.matmul(
                                cps[:, pi * 64:(pi + 1) * 64],
                                sel,
                                c_sb[:, bq * 64:(bq + 1) * 64],
                                start=(pi == 0), stop=(pi == 7))
                        crs = crp.tile([128, 512], dt.bfloat16, tag="crs")
                        nc.scalar.copy(crs[:], cps[:])
                        if stage < 5:
                            continue
                        xcol = x_ik[:, g * B + b0:g * B + b0 + 8]
                        xcb = xcol.unsqueeze(2).broadcast_to([128, 8, 64])
                        c3 = crs[:].rearrange("p (pi o) -> p pi o", pi=8)
                        y3 = y[:, b0 * 64:b0 * 64 + 512]
                        y3 = y3.rearrange("p (pi o) -> p pi o", pi=8)
                        nc.vector.tensor_tensor(y3, c3, xcb, ALU.mult)

                    y4 = y[:].rearrange("p (b o) -> p b o", b=32)
                    for o in range(O if stage >= 5 else 0):
                        nc.tensor.matmul(
                            s_out_ap(s_t, o),
                            w1[g][:, o * D:(o + 1) * D],
                            y4[:, :, o],
                            start=False, stop=False,
                            tile_position=s_tile_pos(o))

            bank_bracket(s_t, "close")
            v = allreduce_squash(s_t, t)

        nc.sync.dma_start(out_d[:], v[:])

    nc.compile()
    return nc


def _get_compiled():
    import os
    if "nc" not in _CACHE:
        _CACHE["nc"] = build_nc(
            n_routings=int(os.environ.get("CAPS_ROUTINGS", ROUTINGS)),
            skip_cc=bool(int(os.environ.get("CAPS_SKIP_CC", "0"))),
            stage=int(os.environ.get("CAPS_STAGE", "5")))
    return _CACHE["nc"]


def kernel(x, weight):
    from concourse.bass_utils import run_bass_kernel_spmd
    import os

    x = np.asarray(x, dtype=np.float32)
    weight = np.asarray(weight, dtype=np.float32)

    nc = _get_compiled()
    static = _prep_static()
    in_maps = []
    for core in range(NCORES):
        m = _prep_core_inputs(x, weight, core)
        m.update(static)
        in_maps.append(m)

    trace = bool(int(os.environ.get("CAPS_TRACE", "0")))
    res = run_bass_kernel_spmd(nc, in_maps, core_ids=list(range(NCORES)),
                               trace=trace)
    _CACHE["last_result"] = res
    raw = res.results[0]["out_raw"]          # [128, 512]
    # out[b, o, d] = raw[(o%4)*32 + d, (o//4)*32 + b]
    r = raw.reshape(4, 32, 16, 32)            # [o_lo, d, og, b]
    out = r.transpose(3, 2, 0, 1).reshape(B, O, D)
    return np.ascontiguousarray(out.astype(np.float32))


if __name__ == "__main__":
    rng = np.random.default_rng(0)
    x = rng.standard_normal((B, IN_N, K), dtype=np.float32)
    w = (0.01 * rng.standard_normal((O, IN_N, D, K))).astype(np.float32)
    out = kernel(x=x, weight=w)
    print(out.shape, out.dtype)
